# revision 3
# baseline (speedup 1.0000x reference)
"""Trainium2 Bass kernel for nn_CorrProductBlock (equivariant product basis block).

Node-parallel across 8 NeuronCores. Self-contained: hardcodes shapes/sharding.
"""

import numpy as np
import ml_dtypes

import concourse.bass as bass
import concourse.bacc as bacc
import concourse.mybir as mybir
import concourse.tile as tile
from concourse.bass_utils import run_bass_kernel_spmd
from concourse.masks import make_identity

MUL = 128
NUM_ELEM = 64
N_CORES = 8
N_NODES = 100000
TILE_N = 512          # nodes per pipeline tile
NSUB = TILE_N // 128  # 128-node subtiles per tile

F32 = mybir.dt.float32
BF16 = mybir.dt.bfloat16
I16 = mybir.dt.int16

IS_EQ = mybir.AluOpType.is_equal
MULT = mybir.AluOpType.mult
ADD = mybir.AluOpType.add


def _build(ntiles: int):
    """Build the per-core Bass program for `ntiles` tiles of TILE_N nodes."""
    per_core = ntiles * TILE_N
    nc = bacc.Bacc(num_devices=N_CORES)

    xf = nc.dram_tensor("xf", [per_core, 512], F32, kind="ExternalInput")
    ei = nc.dram_tensor("ei", [per_core], I16, kind="ExternalInput")
    # weights (bf16, pre-scaled on host)
    wpre0 = nc.dram_tensor("wpre0", [MUL, MUL], BF16, kind="ExternalInput")
    wpre1 = nc.dram_tensor("wpre1", [MUL, MUL], BF16, kind="ExternalInput")
    wco0 = nc.dram_tensor("wco0", [MUL, MUL], BF16, kind="ExternalInput")
    wco1 = nc.dram_tensor("wco1", [MUL, MUL], BF16, kind="ExternalInput")
    wsc0 = nc.dram_tensor("wsc0", [MUL, MUL], BF16, kind="ExternalInput")
    wsc1 = nc.dram_tensor("wsc1", [MUL, MUL], BF16, kind="ExternalInput")
    t10 = nc.dram_tensor("t10", [NUM_ELEM, MUL], BF16, kind="ExternalInput")
    t11 = nc.dram_tensor("t11", [NUM_ELEM, MUL], BF16, kind="ExternalInput")
    t200 = nc.dram_tensor("t200", [NUM_ELEM, MUL], BF16, kind="ExternalInput")
    t211 = nc.dram_tensor("t211", [NUM_ELEM, MUL], BF16, kind="ExternalInput")
    t201 = nc.dram_tensor("t201", [NUM_ELEM, MUL], BF16, kind="ExternalInput")
    y = nc.dram_tensor("y", [per_core, 512], F32, kind="ExternalOutput")

    with tile.TileContext(nc) as tc:
        with (
            tc.tile_pool(name="singles", bufs=1) as singles,
            tc.tile_pool(name="xin", bufs=3) as xin_pool,
            tc.tile_pool(name="xb", bufs=2) as xb_pool,
            tc.tile_pool(name="xt", bufs=2) as xt_pool,
            tc.tile_pool(name="oh", bufs=2) as oh_pool,
            tc.tile_pool(name="ew", bufs=2) as ew_pool,
            tc.tile_pool(name="outp", bufs=3) as out_pool,
            tc.tile_pool(name="ph", bufs=1, space="PSUM") as ph_pool,
            tc.tile_pool(name="pg", bufs=2, space="PSUM") as pg_pool,
            tc.tile_pool(name="pscr", bufs=2, space="PSUM") as pscr_pool,
        ):
            # --- one-time setup ---
            ident = singles.tile([128, 128], BF16)
            make_identity(nc, ident)
            iota = singles.tile([NUM_ELEM, 1], F32)
            nc.gpsimd.iota(
                iota, pattern=[[0, 1]], base=0, channel_multiplier=1,
                allow_small_or_imprecise_dtypes=True,
            )

            def load_w(dram, p, tag):
                t = singles.tile([p, MUL], BF16, tag=tag)
                nc.sync.dma_start(out=t, in_=dram[:, :])
                return t

            W_pre0 = load_w(wpre0, 128, "wpre0")
            W_pre1 = load_w(wpre1, 128, "wpre1")
            W_co0 = load_w(wco0, 128, "wco0")
            W_co1 = load_w(wco1, 128, "wco1")
            W_sc0 = load_w(wsc0, 128, "wsc0")
            W_sc1 = load_w(wsc1, 128, "wsc1")
            T_10 = load_w(t10, 64, "t10")
            T_11 = load_w(t11, 64, "t11")
            T_200 = load_w(t200, 64, "t200")
            T_211 = load_w(t211, 64, "t211")
            T_201 = load_w(t201, 64, "t201")

            xf_t = xf.rearrange("(t s p) f -> t s p f", s=NSUB, p=128)
            y_t = y.rearrange("(t s p) f -> t s p f", s=NSUB, p=128)

            for it in range(ntiles):
                j0 = it * TILE_N
                # ---- load x tile (node-major) ----
                x_in = xin_pool.tile([128, NSUB, 512], F32, tag="xin")
                nc.sync.dma_start(
                    out=x_in,
                    in_=xf_t[it].rearrange("s p f -> p s f"),
                )
                # elems broadcast [64, 512]
                eb = ew_pool.tile([NUM_ELEM, TILE_N], I16, tag="eb")
                nc.gpsimd.dma_start(
                    out=eb,
                    in_=bass.AP(tensor=ei, offset=j0, ap=[[0, NUM_ELEM], [1, TILE_N]]),
                )
                oh = oh_pool.tile([NUM_ELEM, TILE_N], BF16, tag="oh")
                nc.vector.tensor_scalar(
                    out=oh, in0=eb, scalar1=iota, scalar2=None, op0=IS_EQ
                )

                # ---- cast to bf16 ----
                xb = xb_pool.tile([128, NSUB, 512], BF16, tag="xb")
                nc.gpsimd.tensor_copy(out=xb, in_=x_in)

                # ---- transposes: xb [node, f] -> xT [f-ish, node] ----
                # xT[:, 0, :]  = x0T   [c, n]
                # xT[:, 1+i, :] = x1T_i [c, n]
                xT = xt_pool.tile([128, 4, TILE_N], BF16, tag="xT")
                for sub in range(NSUB):
                    tp = pscr_pool.tile([128, 4, 128], BF16, tag="scratch")
                    xb_s = xb[:, sub, :]
                    nc.tensor.transpose(tp[:, 0, :], xb_s[:, 0:128], ident)
                    xb_vec = xb_s[:, 128:512].rearrange(
                        "p (c three) -> p three c", three=3
                    )
                    for i3 in range(3):
                        nc.tensor.transpose(tp[:, 1 + i3, :], xb_vec[:, i3, :], ident)
                    nc.scalar.copy(
                        out=xT[:, :, sub * 128:(sub + 1) * 128], in_=tp
                    )

                # ---- pre-layer matmuls -> h PSUM [c, path, n] ----
                h = ph_pool.tile([128, 4, TILE_N], F32, tag="h")
                nc.tensor.matmul(h[:, 0, :], W_pre0, xT[:, 0, :], start=True, stop=True)
                for i3 in range(3):
                    nc.tensor.matmul(
                        h[:, 1 + i3, :], W_pre1, xT[:, 1 + i3, :], start=True, stop=True
                    )

                # ---- evacuate h -> bf16 SBUF ----
                c0 = ew_pool.tile([128, TILE_N], BF16, tag="c0")
                nc.scalar.copy(out=c0, in_=h[:, 0, :])
                c1 = ew_pool.tile([128, 3, TILE_N], BF16, tag="c1")
                nc.scalar.copy(out=c1, in_=h[:, 1:4, :])

                # ss = sum_i h1_i^2
                sq = ew_pool.tile([128, 3, TILE_N], BF16, tag="sq")
                nc.vector.tensor_mul(sq, c1, c1)
                ss = ew_pool.tile([128, TILE_N], BF16, tag="ss")
                nc.vector.tensor_add(ss, sq[:, 0, :], sq[:, 1, :])
                ss2 = ew_pool.tile([128, TILE_N], BF16, tag="ss2")
                nc.vector.tensor_add(ss2, ss, sq[:, 2, :])

                # ---- gathers + products ----
                # a1 = (g11 + g201*h0) * h1
                g201 = pg_pool.tile([128, TILE_N], F32, tag="g")
                nc.tensor.matmul(g201, T_201, oh, start=True, stop=True)
                p1 = ew_pool.tile([128, TILE_N], BF16, tag="p1")
                nc.vector.tensor_tensor(out=p1, in0=g201, in1=c0, op=MULT)
                g11 = pg_pool.tile([128, TILE_N], F32, tag="g")
                nc.tensor.matmul(g11, T_11, oh, start=True, stop=True)
                p2 = ew_pool.tile([128, TILE_N], BF16, tag="p2")
                nc.vector.tensor_tensor(out=p2, in0=g11, in1=p1, op=ADD)
                a1 = ew_pool.tile([128, 3, TILE_N], BF16, tag="a1")
                p2b = bass.AP(
                    tensor=p2.tensor, offset=p2.offset,
                    ap=[p2.ap[0], [0, 3], p2.ap[1]],
                )
                nc.vector.tensor_tensor(out=a1, in0=p2b, in1=c1, op=MULT)

                # a0 = h0*(g10 + g200*h0) + g211*ss
                g200 = pg_pool.tile([128, TILE_N], F32, tag="g")
                nc.tensor.matmul(g200, T_200, oh, start=True, stop=True)
                t1 = ew_pool.tile([128, TILE_N], BF16, tag="t1")
                nc.vector.tensor_tensor(out=t1, in0=g200, in1=c0, op=MULT)
                g10 = pg_pool.tile([128, TILE_N], F32, tag="g")
                nc.tensor.matmul(g10, T_10, oh, start=True, stop=True)
                t2 = ew_pool.tile([128, TILE_N], BF16, tag="t2")
                nc.vector.tensor_tensor(out=t2, in0=g10, in1=t1, op=ADD)
                a0a = ew_pool.tile([128, TILE_N], BF16, tag="a0a")
                nc.gpsimd.tensor_tensor(out=a0a, in0=c0, in1=t2, op=MULT)
                g211 = pg_pool.tile([128, TILE_N], F32, tag="g")
                nc.tensor.matmul(g211, T_211, oh, start=True, stop=True)
                z = ew_pool.tile([128, TILE_N], BF16, tag="z")
                nc.vector.tensor_tensor(out=z, in0=g211, in1=ss2, op=MULT)
                a0 = ew_pool.tile([128, TILE_N], BF16, tag="a0")
                nc.gpsimd.tensor_tensor(out=a0, in0=a0a, in1=z, op=ADD)

                # ---- final layer: u = a @ Wco + x @ Wsc  (node-major PSUM) ----
                out_sb = out_pool.tile([128, NSUB, 512], F32, tag="out")
                for sub in range(NSUB):
                    u = pscr_pool.tile([128, 512], F32, tag="scratch")
                    ns = slice(sub * 128, (sub + 1) * 128)
                    nc.tensor.matmul(
                        u[:, 0:128], xT[:, 0, ns], W_sc0, start=True, stop=False
                    )
                    nc.tensor.matmul(
                        u[:, 0:128], a0[:, ns], W_co0, start=False, stop=True
                    )
                    u_vec = u[:, 128:512].rearrange("p (c three) -> p three c", three=3)
                    for i3 in range(3):
                        nc.tensor.matmul(
                            u_vec[:, i3, :], xT[:, 1 + i3, ns], W_sc1,
                            start=True, stop=False,
                        )
                        nc.tensor.matmul(
                            u_vec[:, i3, :], a1[:, i3, ns], W_co1,
                            start=False, stop=True,
                        )
                    nc.scalar.copy(out=out_sb[:, sub, :], in_=u)

                nc.sync.dma_start(
                    out=y_t[it].rearrange("s p f -> p s f"), in_=out_sb
                )

    nc.compile()
    return nc


def _prep_weights(inp):
    s = 1.0 / np.sqrt(MUL)
    s3 = 1.0 / np.sqrt(3.0)
    f = lambda a: np.asarray(a, dtype=np.float32)
    bf = lambda a: np.ascontiguousarray(a.astype(ml_dtypes.bfloat16))
    w = {}
    w["wpre0"] = bf(f(inp["Wpre0"]) * s)
    w["wpre1"] = bf(f(inp["Wpre1"]) * s)
    w["wco0"] = bf((f(inp["Wprod0"]) @ f(inp["Wout0"])) * (s * s))
    w["wco1"] = bf((f(inp["Wprod1"]) @ f(inp["Wout1"])) * (s * s))
    w["wsc0"] = bf(f(inp["Wsc0"]) * s)
    w["wsc1"] = bf(f(inp["Wsc1"]) * s)
    w["t10"] = bf(f(inp["w1_0"]))
    w["t11"] = bf(f(inp["w1_1"]))
    w["t200"] = bf(f(inp["w2_00"]))
    w["t211"] = bf(f(inp["w2_11"]) * s3)
    w["t201"] = bf(f(inp["w2_01"]))
    return w


_cache = {}


def _get_program(ntiles):
    if ntiles not in _cache:
        _cache[ntiles] = _build(ntiles)
    return _cache[ntiles]


def run_sharded(node_feats, node_elems, weights, n_nodes, trace=False):
    """Run on hardware: shard `n_nodes` across 8 cores, pad to tile multiple."""
    per_core_raw = (n_nodes + N_CORES - 1) // N_CORES
    ntiles = (per_core_raw + TILE_N - 1) // TILE_N
    per_core = ntiles * TILE_N

    feats = np.zeros((N_CORES, per_core, 512), dtype=np.float32)
    elems = np.zeros((N_CORES, per_core), dtype=np.int16)
    counts = []
    for c in range(N_CORES):
        lo = c * per_core_raw
        hi = min(n_nodes, lo + per_core_raw)
        cnt = max(0, hi - lo)
        counts.append(cnt)
        if cnt:
            feats[c, :cnt] = node_feats[lo:hi]
            elems[c, :cnt] = node_elems[lo:hi].astype(np.int16)

    nc = _get_program(ntiles)
    in_maps = [
        {"xf": feats[c], "ei": elems[c], **weights} for c in range(N_CORES)
    ]
    res = run_bass_kernel_spmd(
        nc, in_maps, core_ids=list(range(N_CORES)), trace=trace
    )
    out = np.empty((n_nodes, 512), dtype=np.float32)
    for c in range(N_CORES):
        lo = c * per_core_raw
        if counts[c]:
            out[lo:lo + counts[c]] = res.results[c]["y"][:counts[c]]
    return out, res


def kernel(**inputs):
    inputs = {k: np.asarray(v) for k, v in inputs.items()}
    node_feats = inputs["node_feats"].astype(np.float32, copy=False)
    node_elems = inputs["node_elems"]
    weights = _prep_weights(inputs)
    out, _ = run_sharded(node_feats, node_elems, weights, node_feats.shape[0])
    return out


# revision 32
# speedup vs baseline: 246.2737x; 246.2737x over previous
"""Trainium2 Bass kernel for nn_CorrProductBlock (equivariant product basis block).

Node-parallel across 8 NeuronCores. Self-contained: hardcodes shapes/sharding.
"""

import numpy as np
import ml_dtypes

import concourse.bass as bass
import concourse.bacc as bacc
import concourse.mybir as mybir
import concourse.tile as tile
from concourse.bass_utils import run_bass_kernel_spmd
from concourse.masks import make_identity

MUL = 128
NUM_ELEM = 64
N_CORES = 8
N_NODES = 100000
TILE_N = 512          # nodes per pipeline tile
NSUB = TILE_N // 128  # 128-node subtiles per tile

F32 = mybir.dt.float32
BF16 = mybir.dt.bfloat16
I16 = mybir.dt.int16

IS_EQ = mybir.AluOpType.is_equal
MULT = mybir.AluOpType.mult
ADD = mybir.AluOpType.add


def _build(ntiles: int, repeat: int = 1):
    """Build the per-core Bass program for `ntiles` tiles of TILE_N nodes.

    repeat>1 wraps the whole pipeline in a device-side loop (for timing
    amplification only — reprocesses the same data).
    """
    per_core = ntiles * TILE_N
    nc = bacc.Bacc(num_devices=N_CORES, dynamic_dma_scratch_size=98304)

    xf = nc.dram_tensor("xf", [per_core, 512], F32, kind="ExternalInput")
    ohb = nc.dram_tensor("ohb", [NUM_ELEM, per_core], BF16, kind="ExternalInput")
    wpre0 = nc.dram_tensor("wpre0", [MUL, MUL], BF16, kind="ExternalInput")
    wpre1 = nc.dram_tensor("wpre1", [MUL, MUL], BF16, kind="ExternalInput")
    wco0 = nc.dram_tensor("wco0", [MUL, MUL], BF16, kind="ExternalInput")
    wco1 = nc.dram_tensor("wco1", [MUL, MUL], BF16, kind="ExternalInput")
    wsc0 = nc.dram_tensor("wsc0", [MUL, MUL], BF16, kind="ExternalInput")
    wsc1 = nc.dram_tensor("wsc1", [MUL, MUL], BF16, kind="ExternalInput")
    t10 = nc.dram_tensor("t10", [NUM_ELEM, MUL], BF16, kind="ExternalInput")
    t11 = nc.dram_tensor("t11", [NUM_ELEM, MUL], BF16, kind="ExternalInput")
    t200 = nc.dram_tensor("t200", [NUM_ELEM, MUL], BF16, kind="ExternalInput")
    t211 = nc.dram_tensor("t211", [NUM_ELEM, MUL], BF16, kind="ExternalInput")
    t201 = nc.dram_tensor("t201", [NUM_ELEM, MUL], BF16, kind="ExternalInput")
    y = nc.dram_tensor("y", [per_core, 512], F32, kind="ExternalOutput")

    with tile.TileContext(nc) as tc:
        with (
            tc.tile_pool(name="singles", bufs=1) as singles,
            tc.tile_pool(name="xin", bufs=3) as xin_pool,
            tc.tile_pool(name="xb", bufs=4) as xb_pool,
            tc.tile_pool(name="xt", bufs=3) as xt_pool,
            tc.tile_pool(name="oh", bufs=3) as oh_pool,
            tc.tile_pool(name="ew", bufs=3) as ew_pool,
            tc.tile_pool(name="outp", bufs=3) as out_pool,
            tc.tile_pool(name="ph", bufs=1, space="PSUM") as ph_pool,
            tc.tile_pool(name="pg", bufs=1, space="PSUM") as pg_pool,
            tc.tile_pool(name="ptp", bufs=1, space="PSUM") as ptp_pool,
            tc.tile_pool(name="pu", bufs=2, space="PSUM") as pu_pool,
        ):
            # --- one-time setup ---
            ident = singles.tile([128, 128], BF16)
            make_identity(nc, ident)

            def load_w(dram, p, tag):
                t = singles.tile([p, MUL], BF16, tag=tag)
                nc.sync.dma_start(out=t, in_=dram[:, :])
                return t

            W_pre0 = load_w(wpre0, 128, "wpre0")
            W_pre1 = load_w(wpre1, 128, "wpre1")
            W_co0 = load_w(wco0, 128, "wco0")
            W_co1 = load_w(wco1, 128, "wco1")
            W_sc0 = load_w(wsc0, 128, "wsc0")
            W_sc1 = load_w(wsc1, 128, "wsc1")
            T_10 = load_w(t10, 64, "t10")
            T_11 = load_w(t11, 64, "t11")
            T_200 = load_w(t200, 64, "t200")
            T_211 = load_w(t211, 64, "t211")
            T_201 = load_w(t201, 64, "t201")

            xf_t = xf.rearrange("(t s p) f -> t s p f", s=NSUB, p=128)
            y_t = y.rearrange("(t s p) f -> t s p f", s=NSUB, p=128)

            import contextlib
            rep_ctx = (
                tc.For_i(0, repeat, hint_engines=tuple(mybir.ALL_ENGINES))
                if repeat > 1 else contextlib.nullcontext()
            )
            with rep_ctx:
                _tile_body(nc, tc, locals())

    nc.compile()
    return nc


def _tile_body(nc, tc, env):
    """Software-pipelined emission: per-engine streams are FIFO in program
    order, so next-tile early stages are emitted before this-tile late stages
    to keep every engine fed."""
    ntiles = env["ntiles"]
    xf_t, y_t = env["xf_t"], env["y_t"]
    xb_pool, xt_pool = env["xb_pool"], env["xt_pool"]
    oh_pool, ew_pool, out_pool = env["oh_pool"], env["ew_pool"], env["out_pool"]
    ph_pool, pg_pool = env["ph_pool"], env["pg_pool"]
    ptp_pool, pu_pool = env["ptp_pool"], env["pu_pool"]
    ident, ohb = env["ident"], env["ohb"]
    W_pre0, W_pre1 = env["W_pre0"], env["W_pre1"]
    W_co0, W_co1 = env["W_co0"], env["W_co1"]
    W_sc0, W_sc1 = env["W_sc0"], env["W_sc1"]
    T_10, T_11 = env["T_10"], env["T_11"]
    T_200, T_211, T_201 = env["T_200"], env["T_211"], env["T_201"]

    st = [dict() for _ in range(ntiles)]  # per-tile live tiles

    def stage_load(i):
        if not (0 <= i < ntiles):
            return
        xb = xb_pool.tile([128, NSUB, 512], BF16, tag="xb")
        nc.gpsimd.dma_start(out=xb, in_=xf_t[i].rearrange("s p f -> p s f"))
        st[i]["xb"] = xb

    def stage_oh(i):
        if not (0 <= i < ntiles):
            return
        j0 = i * TILE_N
        oh = oh_pool.tile([NUM_ELEM, TILE_N], BF16, tag="oh")
        nc.scalar.dma_start(out=oh, in_=ohb[:, j0:j0 + TILE_N])
        st[i]["oh"] = oh

    def stage_tp(i):
        # transposes + xT copies
        if not (0 <= i < ntiles):
            return
        xb = st[i]["xb"]
        xT = xt_pool.tile([128, 4, TILE_N], BF16, tag="xT")
        for sub in range(NSUB):
            tp = ptp_pool.tile([128, 4, 128], BF16, tag="tp")
            xb_s = xb[:, sub, :]
            nc.tensor.transpose(tp[:, 0, :], xb_s[:, 0:128], ident)
            xb_vec = xb_s[:, 128:512].rearrange("p (c three) -> p three c", three=3)
            for i3 in range(3):
                nc.tensor.transpose(tp[:, 1 + i3, :], xb_vec[:, i3, :], ident)
            xT_dst = xT[:, :, sub * 128:(sub + 1) * 128]
            if sub < 2:
                nc.scalar.copy(out=xT_dst, in_=tp)
            else:
                nc.vector.tensor_copy(out=xT_dst, in_=tp)
        st[i]["xT"] = xT

    def stage_pre(i):
        if not (0 <= i < ntiles):
            return
        xT = st[i]["xT"]
        h = ph_pool.tile([128, 4, TILE_N], F32, tag="h")
        nc.tensor.matmul(h[:, 0, :], W_pre0, xT[:, 0, :], start=True, stop=True)
        for i3 in range(3):
            nc.tensor.matmul(
                h[:, 1 + i3, :], W_pre1, xT[:, 1 + i3, :], start=True, stop=True
            )
        st[i]["h"] = h

    def stage_evac(i):
        if not (0 <= i < ntiles):
            return
        h = st[i]["h"]
        c0 = ew_pool.tile([128, TILE_N], BF16, tag="c0")
        nc.scalar.copy(out=c0, in_=h[:, 0, :])
        c1 = ew_pool.tile([128, 3, TILE_N], BF16, tag="c1")
        nc.scalar.copy(out=c1, in_=h[:, 1:4, :])
        st[i]["c0"], st[i]["c1"] = c0, c1

    def stage_sq(i):
        if not (0 <= i < ntiles):
            return
        c1 = st[i]["c1"]
        sq = ew_pool.tile([128, 3, TILE_N], BF16, tag="sq")
        nc.vector.tensor_mul(sq, c1, c1)
        ss = ew_pool.tile([128, TILE_N], BF16, tag="ss")
        nc.gpsimd.tensor_add(ss, sq[:, 0, :], sq[:, 1, :])
        ss2 = ew_pool.tile([128, TILE_N], BF16, tag="ss2")
        nc.gpsimd.tensor_add(ss2, ss, sq[:, 2, :])
        st[i]["ss2"] = ss2

    def stage_gather(i):
        if not (0 <= i < ntiles):
            return
        oh, c0, c1, ss2 = st[i]["oh"], st[i]["c0"], st[i]["c1"], st[i]["ss2"]
        # a1 = (g11 + g201*h0) * h1
        g201 = pg_pool.tile([128, TILE_N], F32, tag="g")
        nc.tensor.matmul(g201, T_201, oh, start=True, stop=True)
        p1 = ew_pool.tile([128, TILE_N], BF16, tag="p1")
        nc.vector.tensor_tensor(out=p1, in0=g201, in1=c0, op=MULT)
        g11 = pg_pool.tile([128, TILE_N], F32, tag="g")
        nc.tensor.matmul(g11, T_11, oh, start=True, stop=True)
        p2 = ew_pool.tile([128, TILE_N], BF16, tag="p2")
        nc.vector.tensor_tensor(out=p2, in0=g11, in1=p1, op=ADD)
        a1 = ew_pool.tile([128, 3, TILE_N], BF16, tag="a1")
        p2b = bass.AP(
            tensor=p2.tensor, offset=p2.offset,
            ap=[p2.ap[0], [0, 3], p2.ap[1]],
        )
        nc.vector.tensor_tensor(out=a1, in0=p2b, in1=c1, op=MULT)
        # a0 = h0*(g10 + g200*h0) + g211*ss
        g200 = pg_pool.tile([128, TILE_N], F32, tag="g")
        nc.tensor.matmul(g200, T_200, oh, start=True, stop=True)
        t1 = ew_pool.tile([128, TILE_N], BF16, tag="t1")
        nc.vector.tensor_tensor(out=t1, in0=g200, in1=c0, op=MULT)
        g10 = pg_pool.tile([128, TILE_N], F32, tag="g")
        nc.tensor.matmul(g10, T_10, oh, start=True, stop=True)
        t2 = ew_pool.tile([128, TILE_N], BF16, tag="t2")
        nc.vector.tensor_tensor(out=t2, in0=g10, in1=t1, op=ADD)
        a0a = ew_pool.tile([128, TILE_N], BF16, tag="a0a")
        nc.gpsimd.tensor_tensor(out=a0a, in0=c0, in1=t2, op=MULT)
        g211 = pg_pool.tile([128, TILE_N], F32, tag="g")
        nc.tensor.matmul(g211, T_211, oh, start=True, stop=True)
        z = ew_pool.tile([128, TILE_N], BF16, tag="z")
        nc.vector.tensor_tensor(out=z, in0=g211, in1=ss2, op=MULT)
        st[i]["a0a"], st[i]["z"], st[i]["a1"] = a0a, z, a1

    def stage_final_mm(i):
        if not (0 <= i < ntiles):
            return
        xT, a1 = st[i]["xT"], st[i]["a1"]
        a0a, z = st[i]["a0a"], st[i]["z"]
        us = []
        for sub in range(NSUB):
            u = pu_pool.tile([128, 512], F32, tag="u")
            ns = slice(sub * 128, (sub + 1) * 128)
            nc.tensor.matmul(u[:, 0:128], xT[:, 0, ns], W_sc0, start=True, stop=False)
            nc.tensor.matmul(u[:, 0:128], a0a[:, ns], W_co0, start=False, stop=False)
            nc.tensor.matmul(u[:, 0:128], z[:, ns], W_co0, start=False, stop=True)
            u_vec = u[:, 128:512].rearrange("p (c three) -> p three c", three=3)
            for i3 in range(3):
                nc.tensor.matmul(
                    u_vec[:, i3, :], xT[:, 1 + i3, ns], W_sc1,
                    start=True, stop=False,
                )
                nc.tensor.matmul(
                    u_vec[:, i3, :], a1[:, i3, ns], W_co1,
                    start=False, stop=True,
                )
            us.append(u)
        st[i]["us"] = us

    def stage_ucopy(i):
        if not (0 <= i < ntiles):
            return
        us = st[i]["us"]
        out_sb = out_pool.tile([128, NSUB, 512], F32, tag="out")
        for sub in range(NSUB):
            if sub < 3:
                nc.scalar.copy(out=out_sb[:, sub, :], in_=us[sub])
            else:
                nc.vector.tensor_copy(out=out_sb[:, sub, :], in_=us[sub])
        st[i]["out_sb"] = out_sb

    def stage_out(i):
        if not (0 <= i < ntiles):
            return
        nc.sync.dma_start(
            out=y_t[i].rearrange("s p f -> p s f"), in_=st[i]["out_sb"]
        )
        st[i].clear()

    # prologue
    stage_load(0)
    stage_load(1)
    stage_oh(0)
    stage_tp(0)
    stage_pre(0)
    for i in range(ntiles + 1):
        stage_load(i + 2)
        stage_final_mm(i - 1)
        stage_oh(i + 1)
        stage_tp(i + 1)
        stage_evac(i)
        stage_ucopy(i - 1)
        stage_out(i - 1)
        stage_sq(i)
        stage_gather(i)
        stage_pre(i + 1)


def _prep_weights(inp):
    s = 1.0 / np.sqrt(MUL)
    s3 = 1.0 / np.sqrt(3.0)
    f = lambda a: np.asarray(a, dtype=np.float32)
    bf = lambda a: np.ascontiguousarray(a.astype(ml_dtypes.bfloat16))
    w = {}
    w["wpre0"] = bf(f(inp["Wpre0"]) * s)
    w["wpre1"] = bf(f(inp["Wpre1"]) * s)
    w["wco0"] = bf((f(inp["Wprod0"]) @ f(inp["Wout0"])) * (s * s))
    w["wco1"] = bf((f(inp["Wprod1"]) @ f(inp["Wout1"])) * (s * s))
    w["wsc0"] = bf(f(inp["Wsc0"]) * s)
    w["wsc1"] = bf(f(inp["Wsc1"]) * s)
    w["t10"] = bf(f(inp["w1_0"]))
    w["t11"] = bf(f(inp["w1_1"]))
    w["t200"] = bf(f(inp["w2_00"]))
    w["t211"] = bf(f(inp["w2_11"]) * s3)
    w["t201"] = bf(f(inp["w2_01"]))
    return w


_cache = {}


def _get_program(ntiles):
    if ntiles not in _cache:
        _cache[ntiles] = _build(ntiles)
    return _cache[ntiles]


def run_sharded(node_feats, node_elems, weights, n_nodes, trace=False):
    """Run on hardware: shard `n_nodes` across 8 cores, pad to tile multiple."""
    per_core_raw = (n_nodes + N_CORES - 1) // N_CORES
    ntiles = (per_core_raw + TILE_N - 1) // TILE_N
    per_core = ntiles * TILE_N

    feats = np.zeros((N_CORES, per_core, 512), dtype=np.float32)
    ohb = np.zeros((N_CORES, NUM_ELEM, per_core), dtype=ml_dtypes.bfloat16)
    counts = []
    for c in range(N_CORES):
        lo = c * per_core_raw
        hi = min(n_nodes, lo + per_core_raw)
        cnt = max(0, hi - lo)
        counts.append(cnt)
        if cnt:
            feats[c, :cnt] = node_feats[lo:hi]
            e = np.asarray(node_elems[lo:hi]).astype(np.int64)
            ohb[c, e, np.arange(cnt)] = 1.0

    nc = _get_program(ntiles)
    in_maps = [
        {"xf": feats[c], "ohb": ohb[c], **weights} for c in range(N_CORES)
    ]
    res = run_bass_kernel_spmd(
        nc, in_maps, core_ids=list(range(N_CORES)), trace=trace
    )
    out = np.empty((n_nodes, 512), dtype=np.float32)
    for c in range(N_CORES):
        lo = c * per_core_raw
        if counts[c]:
            out[lo:lo + counts[c]] = res.results[c]["y"][:counts[c]]
    return out, res


def kernel(**inputs):
    inputs = {k: np.asarray(v) for k, v in inputs.items()}
    node_feats = inputs["node_feats"].astype(np.float32, copy=False)
    node_elems = inputs["node_elems"]
    weights = _prep_weights(inputs)
    out, _ = run_sharded(node_feats, node_elems, weights, node_feats.shape[0])
    return out
